# revision 40
# baseline (speedup 1.0000x reference)
"""Trainium2 Bass kernel for nn_AdaptiveTransformerModel (gated multi-head
self-attention with per-head scalar normalization), distributed over 8
NeuronCores via head parallelism + AllToAll.

One fused instruction stream, scalar-engine(exp)/PE co-bound steady state:
  - Host passes X pre-transposed ([D, B*T] bf16) so X^T tiles are plain
    contiguous DMAs (v1 spent 164us in serial transposing DMAs gating a
    cold PE through a separate projection phase).
  - Per-core computation (2 heads, all batches), bf16 matmuls / fp32 stats:
    Q^T/K^T/V^T = (X W + b).T laid out [128=(2 heads x 64 hd), B*T]. Per
    chunk (512 q): S^T = K Q^T as a row-tiled pair over the 2 heads
    (concurrent via PE row groups 0-1/2-3), E = exp(S^T/8) in one ACT pass
    over both heads [128, 1024], O^T accumulated as [V | ones].T E (row 64
    = softmax denominators, rows 96 = sumsq via a col-group-96 matmul).
  - QKV projections for batches 1..3, V transposes, and the Wo load are
    interleaved as small deadline-tagged quanta into the attention k-loop,
    filling PE slack under the exp stream; QKV bias adds run on DVE so the
    scalar engine does exp only (plus one tail sqrt).
  - Rotated software pipeline: chunk cc's S(0)/exp(0) issue at the end of
    chunk cc-1 (before its drain) so ACT never idles at chunk boundaries.
  - Chunks processed in natural order; AllToAll piece p carries chunks
    p*8..p*8+7 (slot ci <- chunk p*8+ci), so core ci's output rows are
    chunks {ci, ci+8} (host gather un-permutes). Piece 0 fires at 50% and
    overlaps the stream; piece 1 carries the per-head scales s_h =
    1/max(mean ||O_h||, 1e-5) in an extra row (129 rows) so no separate
    AllGather is needed. Norm stats bounce through DRAM; all sqrt work is
    deferred to one tail pass (single ACT table switch).
  - Final projection P^T = Wo_all^T (s * G) + bo per received piece
    (gate/H folded into Wo on the host); dummy matmuls bridge the A2A-1
    idle window so the PE HAM clock stays warm for the tail matmuls.
"""
import os
import sys

import numpy as np

for _p in ("/root/.axon_site", "/root/.axon_site/_ro/trn_rl_repo", "/opt/trn_rl_repo"):
    if os.path.isdir(_p) and _p not in sys.path:
        sys.path.append(_p)

import ml_dtypes
import concourse.bass as bass
import concourse.bacc as bacc
import concourse.mybir as mybir
import concourse.tile as tile
from concourse import bass_utils
from concourse.bass import ts
from concourse.masks import make_identity

f32 = mybir.dt.float32
bf16 = mybir.dt.bfloat16

AF = mybir.ActivationFunctionType
ALU = mybir.AluOpType
BF16NP = ml_dtypes.bfloat16

# problem shapes (hardcoded per harness contract)
B, T, D, H = 4, 2048, 1024, 16
HD = 64
NCORES = 8


class Cfg:
    def __init__(self, B=B, T=T, D=D, ncores=NCORES):
        self.B, self.T, self.D, self.ncores = B, T, D, ncores
        self.RT = B * T                  # flattened rows
        self.RSLC = self.RT // ncores    # output row slice per core
        self.DCH = D // 128              # contraction chunks for D
        self.TQ = min(512, self.RSLC // 2, T)  # q-chunk width
        self.NQC = T // self.TQ          # q-chunks per batch
        self.NKT = T // 128              # k-tiles per batch
        self.NCH = self.B * self.NQC     # total q-chunks
        assert T % self.TQ == 0 and D % 128 == 0 and self.RT % ncores == 0
        assert self.RSLC // self.TQ == 2 and self.TQ % 128 == 0


def build_body(ctx, tc, cfg, x, wq, wk, wv, bq, bk, bv, wo, bo, out, dbg=None):
    from contextlib import ExitStack
    nc = tc.nc
    DCH, TQ, NKT, NQC, RT = cfg.DCH, cfg.TQ, cfg.NKT, cfg.NQC, cfg.RT
    RW = TQ                              # QKV row-chunk width
    RCPB = cfg.T // RW                   # row-chunks per batch (4)
    NCP = cfg.NCH // 2                   # chunks per a2a piece (8)
    NHALF = cfg.RSLC // 2                # cols per a2a slot (512)
    HCH = (128 * cfg.ncores) // 128      # final contraction chunks (8)

    constp = ctx.enter_context(tc.tile_pool(name="const", bufs=1))
    ident = constp.tile([128, 128], f32)
    make_identity(nc, ident[:])
    ident_bf = constp.tile([128, 128], bf16)
    nc.vector.tensor_copy(ident_bf[:], ident[:])
    ones_f32 = constp.tile([128, 1], f32)
    nc.vector.memset(ones_f32[:], 1.0)
    ones_bf = constp.tile([128, 1], bf16)
    nc.vector.tensor_copy(ones_bf[:], ones_f32[:])
    bq_sb = constp.tile([128, 1], f32)
    bk_sb = constp.tile([128, 1], f32)
    bv_sb = constp.tile([128, 1], f32)
    nc.sync.dma_start(bq_sb[:], bq[:, None])
    nc.sync.dma_start(bk_sb[:], bk[:, None])
    nc.sync.dma_start(bv_sb[:], bv[:, None])
    bo_sb = constp.tile([128, DCH], f32)
    nc.sync.dma_start(bo_sb[:], bo.rearrange("(c p) -> p c", p=128))

    # persistent SBUF buffers (live for the whole kernel; ~81KB/partition
    # total SBUF stays under budget without mid-stream frees)
    persistp = ctx.enter_context(tc.tile_pool(name="persist", bufs=1))
    o_all = persistp.tile([128, RT], bf16, name="o_all")
    qt_all = persistp.tile([128, RT], bf16, name="qt_all")
    kt_all = persistp.tile([128, RT], bf16, name="kt_all")
    vaug = persistp.tile([128, cfg.B, NKT, 2, 65], bf16, name="vaug")
    wo_sb = persistp.tile([128, HCH, cfg.D], bf16, name="wo_sb")
    nc.vector.tensor_copy(
        vaug[:, :, :, :, 64:65],
        ones_f32[:].to_broadcast((128, cfg.B, NKT, 2, 1)))
    wqkvp = ctx.enter_context(tc.tile_pool(name="wqkv", bufs=1))
    wq_sb = wqkvp.tile([128, DCH, 128], bf16)
    wk_sb = wqkvp.tile([128, DCH, 128], bf16)
    wv_sb = wqkvp.tile([128, DCH, 128], bf16)
    nc.sync.dma_start(wq_sb[:], wq.rearrange("(c p) m -> p c m", p=128))
    nc.sync.dma_start(wk_sb[:], wk.rearrange("(c p) m -> p c m", p=128))
    nc.sync.dma_start(wv_sb[:], wv.rearrange("(c p) m -> p c m", p=128))

    dramp = ctx.enter_context(tc.tile_pool(name="dram", bufs=1, space="DRAM"))
    nrm_dram = dramp.tile([2, 2, NCP, TQ], f32)
    sum_dram = dramp.tile([2, 2, NCP, TQ], f32)
    rec_dram = dramp.tile([2, 2, NCP, TQ], f32)
    # piece 1 carries an extra row (128): [s0, s1, 0...] of the sending core
    a2a_rows = [128, 129]
    a2a_in = [dramp.tile([cfg.ncores, a2a_rows[h], NHALF], bf16,
                         name=f"a2a_in{h}") for h in range(2)]
    a2a_out = [dramp.tile([cfg.ncores, a2a_rows[h], NHALF], bf16,
                          name=f"a2a_out{h}")
               for h in range(2)]

    gp = ctx.enter_context(tc.tile_pool(name="g", bufs=1))
    g_all = gp.tile([128, HCH, 2, NHALF], bf16, name="g_all")
    zrow = constp.tile([1, NHALF], bf16)
    nc.vector.memset(zrow[:], 0.0)
    for c in range(cfg.ncores):
        nc.sync.dma_start(a2a_in[1][c, 128:129, :], zrow[0:1, :])
    epool = ctx.enter_context(tc.tile_pool(name="epool", bufs=8))
    ph2 = ctx.enter_context(tc.tile_pool(name="ph2", bufs=2))
    xtp = ctx.enter_context(tc.tile_pool(name="xt", bufs=6))
    vtp = ctx.enter_context(tc.tile_pool(name="vtmp", bufs=2))

    mainps = ExitStack()
    sps = mainps.enter_context(tc.tile_pool(name="sps", bufs=2, space="PSUM"))
    ops = mainps.enter_context(tc.tile_pool(name="ops", bufs=1, space="PSUM"))
    auxps = mainps.enter_context(tc.tile_pool(name="auxps", bufs=2, space="PSUM"))

    # warm the PE HAM clock while the first weight/X DMAs are in flight so
    # the head projections run at 2.4GHz instead of the cold 1.2GHz rate
    warm0 = auxps.tile([128, 128], f32, tag="qkv", bufs=1, name="warm0")
    for _ in range(32):
        nc.tensor.matmul(warm0[:], ident_bf[:], ident_bf[:],
                         start=True, stop=True, skip_group_check=True)

    # ---------------- QKV quanta ----------------
    xt_tiles, qkv_ps, vt_tiles = {}, {}, {}

    def u_xt(rc):
        def f():
            xt = xtp.tile([128, DCH, RW], bf16, tag="xt", name=f"xt{rc}")
            xt_tiles[rc] = xt
            for d in range(DCH):
                nc.sync.dma_start(xt[:, d, :], x[ts(d, 128), ts(rc, RW)])
        return f

    def u_proj_mm(rc, w_sb, d0, d1, proj):
        def f():
            if d0 == 0:
                qkv_ps[(rc, proj)] = auxps.tile(
                    [128, RW], f32, tag="qkv", bufs=1, name=f"ps_{proj}{rc}")
            ps = qkv_ps[(rc, proj)]
            xt = xt_tiles[rc]
            for d in range(d0, d1):
                nc.tensor.matmul(ps[:], w_sb[:, d, :], xt[:, d, :],
                                 start=(d == 0), stop=(d == DCH - 1),
                                 skip_group_check=True)
        return f

    def u_proj_bias(rc, b_sb, dest, proj):
        def f():
            ps = qkv_ps.pop((rc, proj))
            nc.vector.tensor_scalar(out=dest[:, ts(rc, RW)], in0=ps[:],
                                    scalar1=b_sb[:, 0:1], scalar2=None,
                                    op0=ALU.add)
        return f

    def u_v_bias(rc):
        def f():
            ps = qkv_ps.pop((rc, "v"))
            vt = vtp.tile([128, RW], bf16, tag="vt", name=f"vt{rc}")
            vt_tiles[rc] = vt
            nc.vector.tensor_scalar(out=vt[:], in0=ps[:],
                                    scalar1=bv_sb[:, 0:1], scalar2=None,
                                    op0=ALU.add)
        return f

    def u_v_tr(rc, j):
        # transpose one 128-col slab of V for both heads into one PSUM tile
        def f():
            vt = vt_tiles[rc]
            b_idx = (rc * RW) // cfg.T
            kt_idx = ((rc * RW) % cfg.T) // 128 + j
            vp = auxps.tile([128, 2, 64], bf16, tag="vp", bufs=1,
                            name=f"vp{rc}_{j}")
            nc.tensor.transpose(
                vp[:].rearrange("p h c -> p (h c)"), vt[:, ts(j, 128)],
                ident_bf[:])
            nc.vector.tensor_copy(vaug[:, b_idx, kt_idx, :, 0:64], vp[:])
        return f

    def u_wo_load():
        def f():
            nc.sync.dma_start(wo_sb[:], wo.rearrange("(c p) m -> p c m", p=128))
        return f

    def qkv_units_for_rc(rc, proj):
        w_sb, b_sb, dest = {
            "k": (wk_sb, bk_sb, kt_all), "q": (wq_sb, bq_sb, qt_all),
            "v": (wv_sb, bv_sb, None)}[proj]
        us = []
        for d0 in range(0, DCH, 2):
            us.append(u_proj_mm(rc, w_sb, d0, d0 + 2, proj))
        if proj == "v":
            us.append(u_v_bias(rc))
            us += [u_v_tr(rc, j) for j in range(RW // 128)]
        else:
            us.append(u_proj_bias(rc, b_sb, dest, proj))
        return us

    # pending: (deadline_cc, unit) quanta run inside the k-loop, in order.
    # Deadline = first chunk index that needs the unit's output (backstop;
    # normally the in-loop pops drain well ahead of it).
    pending = []
    for i in range(2, RCPB):  # batch 0's later K/V (needed mid-chunk-0)
        dl = (4 * i - 2) / NKT
        pending += [(dl, u) for u in qkv_units_for_rc(i, "k")]
        pending += [(dl, u) for u in qkv_units_for_rc(i, "v")]
    for i in range(1, RCPB):  # batch 0's deferred Q projections (chunk i)
        pending += [(i, u) for u in qkv_units_for_rc(i, "q")]
    for b in range(1, cfg.B):
        dl = b * NQC
        rcs = [b * RCPB + i for i in range(RCPB)]
        rest = {rc: (qkv_units_for_rc(rc, "k") + qkv_units_for_rc(rc, "v")
                     + qkv_units_for_rc(rc, "q")) for rc in rcs}
        merged = [u_xt(rcs[0]), u_xt(rcs[1])]
        for i, rc in enumerate(rcs):
            merged += rest[rc]
            if i + 2 < RCPB:
                merged.append(u_xt(rcs[i + 2]))
        pending += [(dl, u) for u in merged]
        if b == 2:
            pending.append((dl, u_wo_load()))

    # ------------- head: QKV rc0/rc1 K+V and chunk-0's Q ------------------
    for rc in range(RCPB):
        u_xt(rc)()
    for rc in range(2):
        for u in qkv_units_for_rc(rc, "k"):
            u()
        for u in qkv_units_for_rc(rc, "v"):
            u()
    for u in qkv_units_for_rc(0, "q"):
        u()

    # ---------------- norm / normalize / a2a rounds ----------------
    norm_tiles = {}

    def norms_prefetch():
        # load chunks 0..14's stats while chunk 15 still computes
        pcol = cfg.NCH * TQ // 64
        nsb = ph2.tile([128, pcol], f32, tag="nsb", bufs=1, name="nsb")
        rcb = ph2.tile([128, pcol], f32, tag="rcb", bufs=1, name="rcb")
        norm_tiles["nsb"], norm_tiles["rcb"] = nsb, rcb
        pc15 = 15 * TQ // 64  # cols for chunks 0..14
        for hl in range(2):
            nc.sync.dma_start(
                nsb[hl * 64:(hl + 1) * 64, 0:pc15],
                nrm_dram[hl].rearrange("a c q -> (a c q)")[0:15 * TQ]
                .rearrange("(p n) -> p n", p=64))
            nc.sync.dma_start(
                rcb[hl * 64:(hl + 1) * 64, 0:pc15],
                rec_dram[hl].rearrange("a c q -> (a c q)")[0:15 * TQ]
                .rearrange("(p n) -> p n", p=64))

    def norms_tail():
        # Sum over all (b, t) of sqrt(sumsq)*recip per head -> ntot[1, 2].
        # Runs once at the tail (single ACT Sqrt table switch).
        pcol = cfg.NCH * TQ // 64
        pc15 = 15 * TQ // 64
        nsb, rcb = norm_tiles["nsb"], norm_tiles["rcb"]
        for hl in range(2):
            nc.sync.dma_start(
                nsb[hl * 64:(hl + 1) * 64, pc15:pcol],
                nrm_dram[hl].rearrange("a c q -> (a c q)")[15 * TQ:]
                .rearrange("(p n) -> p n", p=64))
            nc.sync.dma_start(
                rcb[hl * 64:(hl + 1) * 64, pc15:pcol],
                rec_dram[hl].rearrange("a c q -> (a c q)")[15 * TQ:]
                .rearrange("(p n) -> p n", p=64))
        nrt = ph2.tile([128, pcol], f32, tag="nrt", name="nrt")
        nc.scalar.activation(nrt[:], nsb[:], AF.Sqrt)
        nc.vector.tensor_tensor(out=nrt[:], in0=nrt[:], in1=rcb[:], op=ALU.mult)
        rsum = ph2.tile([128, 1], f32, tag="rsum", name="rsum")
        nc.vector.tensor_reduce(rsum[:], nrt[:], axis=mybir.AxisListType.X,
                                op=ALU.add)
        ntot = auxps.tile([1, 2], f32, tag="qkv", bufs=1, name="ntot")
        for hl in range(2):
            hs = slice(hl * 64, (hl + 1) * 64)
            nc.tensor.matmul(ntot[0:1, hl:hl + 1], ones_f32[hs, 0:1],
                             rsum[hs, :], start=(hl == 0), stop=(hl == 1),
                             skip_group_check=True)
        return ntot

    def recip_chunks(par, ci_lo, ci_hi):
        # batch reciprocal of softmax sums for chunks par*8+[ci_lo,ci_hi)
        pcol = (ci_hi - ci_lo) * TQ // 64
        ssb = ph2.tile([128, pcol], f32, tag="ssb", name="ssb")
        for hl in range(2):
            nc.sync.dma_start(
                ssb[hl * 64:(hl + 1) * 64, :],
                sum_dram[hl, par, ci_lo:ci_hi, :]
                .rearrange("c q -> (c q)").rearrange("(p n) -> p n", p=64))
        rcp = ph2.tile([128, pcol], f32, tag="rcp", name="rcp")
        nc.vector.reciprocal(rcp[:], ssb[:])
        for hl in range(2):
            nc.sync.dma_start(
                rec_dram[hl, par, ci_lo:ci_hi, :]
                .rearrange("c q -> (c q)").rearrange("(p n) -> p n", p=64),
                rcp[hl * 64:(hl + 1) * 64, :])

    def normalize_chunks(par, ci_lo, ci_hi, fire_piece, recip=True):
        # normalize o_all columns of those chunks; stage their A2A slots
        if recip:
            recip_chunks(par, ci_lo, ci_hi)
        for ci in range(ci_lo, ci_hi):
            cb = (par * NCP + ci) * TQ
            rb = ph2.tile([128, TQ], f32, tag="rb", name="rb")
            for hl in range(2):
                nc.sync.dma_start(
                    rb[hl * 64:(hl + 1) * 64, :],
                    rec_dram[hl, par, ci, :][None, :].to_broadcast((64, TQ)))
            nc.vector.tensor_tensor(out=o_all[:, cb:cb + TQ],
                                    in0=o_all[:, cb:cb + TQ], in1=rb[:],
                                    op=ALU.mult)
            nc.sync.dma_start(a2a_in[par][ci, 0:128, 0:TQ],
                              o_all[:, cb:cb + TQ])
        if fire_piece:
            nc.gpsimd.collective_compute(
                "AllToAll", ALU.bypass,
                replica_groups=[list(range(cfg.ncores))],
                ins=[a2a_in[par][:].opt()], outs=[a2a_out[par][:].opt()])

    # ---------------- fused attention stream ----------------
    # Rotated software pipeline: chunk cc's S(0)/exp(0) are emitted at the
    # end of chunk cc-1's body (before its drain) so ACT never idles across
    # chunk boundaries; the k-loop body then runs t=1..15.
    def mk_flush(b, o_ps):
        def flush_o(te, e_tile):
            for hl in range(2):
                nc.tensor.matmul(o_ps[hl][0:65, :], vaug[:, b, te, hl, :],
                                 e_tile[:, ts(hl, TQ)],
                                 start=(te == 0), stop=(te == NKT - 1),
                                 skip_group_check=True)
        return flush_o

    def s_exp(cc, t):
        b = cc // NQC
        k0 = b * cfg.T + t * 128
        c0 = cc * TQ
        s_pair = sps.tile([128, 2 * TQ], f32, tag="s", name="s_pair")
        for hl in range(2):
            hs = slice(hl * 64, (hl + 1) * 64)
            nc.tensor.matmul(s_pair[:, ts(hl, TQ)],
                             kt_all[hs, k0:k0 + 128],
                             qt_all[hs, c0:c0 + TQ],
                             start=True, stop=True)
        e_pair = epool.tile([128, 2 * TQ], bf16, tag="e", name="e_pair")
        nc.scalar.activation(e_pair[:], s_pair[:], AF.Exp, scale=0.125)
        return e_pair

    def preamble(cc):
        # backstop: pops everything chunk cc needs, then S(0)/exp(0)
        while pending and pending[0][0] <= cc:
            pending.pop(0)[1]()
        o_ps = [ops.tile([128, TQ], f32, tag=f"o{hl}", name=f"o_ps{hl}")
                for hl in range(2)]
        return o_ps, s_exp(cc, 0)

    def drain(cc, o_ps):
        # o_all <- unnormalized O; sumsq row via ones^T (o*o);
        # softmax-sum row (o_ps row 64) and sumsq (row 96) -> DRAM
        c0 = cc * TQ
        sq = ph2.tile([128, TQ], bf16, tag="sq", name="sq")
        nc.vector.tensor_copy(o_all[64:128, c0:c0 + TQ], o_ps[1][0:64, :])
        nc.vector.tensor_copy(o_all[0:64, c0:c0 + TQ], o_ps[0][0:64, :])
        nc.vector.tensor_tensor(out=sq[:, :],
                                in0=o_all[:, c0:c0 + TQ],
                                in1=o_all[:, c0:c0 + TQ], op=ALU.mult)
        par, ci = divmod(cc, NCP)
        for hl in range(2):
            srow = ph2.tile([128, TQ], f32, tag="srow", name="srow")
            nc.vector.tensor_copy(srow[0:1, :], o_ps[hl][64:65, :])
            nc.sync.dma_start(sum_dram[hl, par, ci, :][None, :], srow[0:1, :])
        for hl in range(2):
            hs = slice(hl * 64, (hl + 1) * 64)
            nc.tensor.matmul(o_ps[hl][96:97, :], ones_bf[hs, 0:1],
                             sq[hs, :], start=True, stop=True,
                             skip_group_check=True,
                             tile_position=(hl * 64, 96))
        for hl in range(2):
            nqs = ph2.tile([128, TQ], f32, tag="nqs", name="nqs")
            nc.vector.tensor_copy(nqs[0:1, :], o_ps[hl][96:97, :])
            nc.sync.dma_start(nrm_dram[hl, par, ci, :][None, :], nqs[0:1, :])

    o_ps, prev_e = preamble(0)
    for cc in range(cfg.NCH):
        b = cc // NQC
        flush_o = mk_flush(b, o_ps)
        for t in range(1, NKT):
            while pending and pending[0][0] <= cc + t / NKT:
                pending.pop(0)[1]()
            e_next = s_exp(cc, t)
            flush_o(t - 1, prev_e)
            prev_e = e_next
            if t == 1:
                if cc == NCP // 2:     # chunks 0..3 done -> first half of p0
                    normalize_chunks(0, 0, NCP // 2, False)
                if cc == NCP:          # chunks 4..7 done -> fire piece 0
                    normalize_chunks(0, NCP // 2, NCP, True)
                    nc.sync.dma_start(
                        g_all[:, :, 0, :],
                        a2a_out[0].rearrange("c p q -> p c q"))
                if cc == NCP + NCP // 2:  # chunks 8..11 done
                    normalize_chunks(1, 0, NCP // 2, False)
                if cc == cfg.NCH - 1:  # chunks 12..14 done
                    normalize_chunks(1, NCP // 2, NCP - 1, False)
            if t == 3 and cc == cfg.NCH - 1:
                norms_prefetch()
            # interleave QKV quanta under the exp-saturated ACT; deadline-
            # aware so a batch's projections finish before its first chunk
            if pending:
                need = 0
                for dl, _u in pending:
                    if dl <= cc + 2:
                        need += 1
                    else:
                        break
                slots = 2 * NKT - t
                budget = max(1, -(-need // slots)) if need else \
                    (1 if t % 2 == 0 else 0)
                for _ in range(budget):
                    if pending:
                        pending.pop(0)[1]()
        flush_o(NKT - 1, prev_e)
        old_o_ps = o_ps
        if cc + 1 < cfg.NCH:
            o_ps, prev_e = preamble(cc + 1)
        drain(cc, old_o_ps)
    recip_chunks(1, NCP - 1, NCP)
    ntot = norms_tail()

    # per-head scale s_h = 1/max(ntot/RT, 1e-5); rides row 128 of A2A piece 1
    s_sb = ph2.tile([128, 8], f32, tag="s", name="s_sb")
    nc.vector.memset(s_sb[0:1, :], 0.0)
    nc.vector.tensor_scalar(out=s_sb[0:1, 0:2], in0=ntot[0:1, :],
                            scalar1=1.0 / RT, scalar2=1e-5, op0=ALU.mult,
                            op1=ALU.max)
    nc.vector.reciprocal(s_sb[0:1, 0:2], s_sb[0:1, 0:2])
    s_bf = ph2.tile([1, 16], bf16, tag="sbf", name="s_bf")
    nc.vector.memset(s_bf[:], 0.0)
    nc.vector.tensor_copy(s_bf[0:1, 0:8], s_sb[0:1, :])
    for c in range(cfg.ncores):
        nc.sync.dma_start(a2a_in[1][c, 128:129, 0:16], s_bf[0:1, :])
    normalize_chunks(1, NCP - 1, NCP, False, recip=False)
    nc.gpsimd.collective_compute(
        "AllToAll", ALU.bypass,
        replica_groups=[list(range(cfg.ncores))],
        ins=[a2a_in[1][:].opt()], outs=[a2a_out[1][:].opt()])
    mainps.close()

    if dbg is not None:
        nc.sync.dma_start(dbg["dbg_o"], o_all[:])
        for h in range(2):
            nc.sync.dma_start(
                dbg["dbg_a2a"].rearrange("a (h q) -> h a q", h=2)[h],
                a2a_out[h][:, 0:128, :].rearrange("c p q -> (c p) q"))

    # ---------------- final projection ----------------
    with tc.tile_pool(name="fin", bufs=1) as finp, \
         tc.tile_pool(name="pps", bufs=2, space="PSUM") as pps, \
         tc.tile_pool(name="pout", bufs=3) as poutp:
        # keep the PE's HAM clock warm across the A2A-1 idle window
        warm = pps.tile([128, NHALF], f32, tag="pps", name="warm")
        for _ in range(56):
            nc.tensor.matmul(warm[:], wo_sb[:, 0, 0:128],
                             g_all[:, 0, 0, :], start=True, stop=True,
                             skip_group_check=True)
        # svec[p, j]: per-partition scale for G block j (heads 2j / 2j+1)
        svec_bf = finp.tile([128, HCH], bf16)
        for hl in range(2):
            nc.sync.dma_start(
                svec_bf[hl * 64:(hl + 1) * 64, :],
                a2a_out[1][:, 128, hl][None, :].to_broadcast((64, HCH)))
        svec = finp.tile([128, HCH], f32)
        nc.vector.tensor_copy(svec[:], svec_bf[:])
        nc.sync.dma_start(g_all[:, :, 1, :],
                          a2a_out[1][:, 0:128, :].rearrange("c p q -> p c q"))
        for h in range(2):
            # h=0 scales+matmuls depend only on svec + piece 0 (loaded at
            # 50%), so they run while the piece-1 G load is still in flight
            for j in range(HCH):
                nc.vector.tensor_scalar(out=g_all[:, j, h, :],
                                        in0=g_all[:, j, h, :],
                                        scalar1=svec[:, j:j + 1], scalar2=None,
                                        op0=ALU.mult)
            for dsub in range(DCH):
                ps = pps.tile([128, NHALF], f32, tag="pps", name="pps")
                for j in range(HCH):
                    nc.tensor.matmul(ps[:], wo_sb[:, j, ts(dsub, 128)],
                                     g_all[:, j, h, :],
                                     start=(j == 0), stop=(j == HCH - 1))
                po = poutp.tile([128, NHALF], f32, tag="po", name="po")
                nc.vector.tensor_scalar(out=po[:], in0=ps[:],
                                        scalar1=bo_sb[:, dsub:dsub + 1],
                                        scalar2=None, op0=ALU.add)
                nc.sync.dma_start(
                    out[ts(dsub, 128), h * NHALF:(h + 1) * NHALF], po[:])


def build_nc(cfg, compile=True, debug_outs=False):
    nc = bacc.Bacc("TRN2", target_bir_lowering=False, debug=False,
                   enable_asserts=False, num_devices=cfg.ncores)
    x = nc.dram_tensor("x", [cfg.D, cfg.RT], bf16, kind="ExternalInput").ap()
    wq = nc.dram_tensor("wq", [cfg.D, 128], bf16, kind="ExternalInput").ap()
    wk = nc.dram_tensor("wk", [cfg.D, 128], bf16, kind="ExternalInput").ap()
    wv = nc.dram_tensor("wv", [cfg.D, 128], bf16, kind="ExternalInput").ap()
    bq = nc.dram_tensor("bq", [128], f32, kind="ExternalInput").ap()
    bk = nc.dram_tensor("bk", [128], f32, kind="ExternalInput").ap()
    bv = nc.dram_tensor("bv", [128], f32, kind="ExternalInput").ap()
    wo = nc.dram_tensor("wo", [128 * cfg.ncores, cfg.D], bf16,
                        kind="ExternalInput").ap()
    bo = nc.dram_tensor("bo", [cfg.D], f32, kind="ExternalInput").ap()
    out = nc.dram_tensor("out", [cfg.D, cfg.RSLC], f32, kind="ExternalOutput").ap()
    dbg = None
    if debug_outs:
        dbg = {
            "dbg_o": nc.dram_tensor("dbg_o", [128, cfg.RT], bf16,
                                    kind="ExternalOutput").ap(),
            "dbg_a2a": nc.dram_tensor("dbg_a2a", [cfg.ncores * 128, cfg.RSLC],
                                      bf16, kind="ExternalOutput").ap(),
        }
    from contextlib import ExitStack
    with tile.TileContext(nc) as tc, ExitStack() as ctx:
        build_body(ctx, tc, cfg, x, wq, wk, wv, bq, bk, bv, wo, bo, out, dbg=dbg)
    if compile:
        nc.compile()
    return nc


def make_in_maps(cfg, inputs, H_total=None):
    """Host-side sharding: per-core input dicts."""
    H_tot = H_total or (2 * cfg.ncores)
    X = np.ascontiguousarray(
        np.asarray(inputs["hidden_states"], np.float32).reshape(cfg.RT, cfg.D).T
    ).astype(BF16NP)
    gate_clip = np.clip(np.asarray(inputs["gate"], np.float32), 0.0, 1.0)
    Wo = np.asarray(inputs["Wo"], np.float32)
    bo = np.asarray(inputs["bo"], np.float32)
    wo_all = np.ascontiguousarray(np.concatenate(
        [Wo[h] * (gate_clip[h] / H_tot) for h in range(H_tot)],
        axis=0)).astype(BF16NP)
    bo_sum = (bo * (gate_clip[:, None] / H_tot)).sum(axis=0).astype(np.float32)
    in_maps = []
    for c in range(cfg.ncores):
        h0, h1 = 2 * c, 2 * c + 1
        m = {
            "x": X,
            "wq": np.concatenate([inputs["Wq"][h0], inputs["Wq"][h1]], axis=1,
                                 dtype=np.float32).astype(BF16NP),
            "wk": np.concatenate([inputs["Wk"][h0], inputs["Wk"][h1]], axis=1,
                                 dtype=np.float32).astype(BF16NP),
            "wv": np.concatenate([inputs["Wv"][h0], inputs["Wv"][h1]], axis=1,
                                 dtype=np.float32).astype(BF16NP),
            "bq": np.concatenate([inputs["bq"][h0], inputs["bq"][h1]],
                                 dtype=np.float32),
            "bk": np.concatenate([inputs["bk"][h0], inputs["bk"][h1]],
                                 dtype=np.float32),
            "bv": np.concatenate([inputs["bv"][h0], inputs["bv"][h1]],
                                 dtype=np.float32),
            "wo": wo_all,
            "bo": bo_sum,
        }
        in_maps.append(m)
    return in_maps


def gather_out(cfg, results):
    """results: list of per-core out_maps -> full [B, T, D].

    Core ci's out cols [0:512) = chunk ci (rows ci*512..), cols [512:1024) =
    chunk ci+8 (rows (ci+8)*512..).
    """
    NHALF = cfg.RSLC // 2
    full = np.empty((cfg.RT, cfg.D), np.float32)
    for ci, r in enumerate(results):
        o = np.asarray(r["out"])  # [D, RSLC]
        full[ci * NHALF:(ci + 1) * NHALF] = o[:, 0:NHALF].T
        full[(ci + cfg.ncores) * NHALF:(ci + cfg.ncores + 1) * NHALF] = \
            o[:, NHALF:].T
    return full.reshape(cfg.B, cfg.T, cfg.D)


_COMPILED = {}


def kernel(**inputs) -> np.ndarray:
    cfg = Cfg()
    key = "full"
    if key not in _COMPILED:
        _COMPILED[key] = build_nc(cfg)
    nc = _COMPILED[key]
    in_maps = make_in_maps(cfg, inputs)
    last_exc = None
    for _attempt in range(3):
        try:
            res = bass_utils.run_bass_kernel_spmd(
                nc, in_maps, core_ids=list(range(cfg.ncores)))
            return gather_out(cfg, res.results)
        except Exception as e:  # transient NRT_EXEC_UNIT_UNRECOVERABLE faults
            last_exc = e
    raise last_exc


if __name__ == "__main__":
    import reference
    inputs = {k: np.asarray(v) for k, v in reference.setup_inputs().items()}
    out = kernel(**inputs)
    exp = np.asarray(reference.reference(**inputs))
    rel = np.linalg.norm(out - exp) / np.linalg.norm(exp)
    print("Relative error:", rel)
